# revision 14
# baseline (speedup 1.0000x reference)
"""Trainium2 Bass kernel for nn_Kernel_6199962355332830965 (sparse_attention).

Reference computation (per batch n, with C=128, H=W=48, HW=2304):
    t1  = max_c x ;  t4 = max(-x, roll(-x,1,w)) = -min(x, roll_w(x))
    t5  = p5_w * t4 ;  t6 = w6 @ t5
    t7  = t6^T x / sqrt(C)
    t8/t9/t10/t11 = dilated conv + unfold + 1x1 conv + channel roll of t1
    t12 = max(x, t11)
    out[n,q,c] = sum_p t7[n,p,q] t12[n,c,p] / sqrt(HW)

Algebraic restructuring (v3):
    out = s * B^T x  with  B = t6 t12^T = w6 (t5 t12^T) = w6 G
  - B = w6 G is one extra 128x128 matmul (A = w6T, moving = Gs); out
    then comes straight from x chunks, so no y tensor or its drains.
  - G[c1,c] = sum_p t5[c1,p] t12[c,p]: accumulated over 18 position-major
    128-blocks, with t5T produced by the DMA XBAR (dma_start_transpose)
    and t12T = max(xT, t11T) where xT also comes from the XBAR.
  - t11T from matmuls of 15 shifted copies of zero-padded t1 against
    K_effT[(3j+k),o] = sum_m w8[m,j] w10[(o-1)%C, 3m+k] (conv+unfold+
    1x1+roll folded into one 15-tap kernel).
  - single-shot matmuls write bf16 straight into PSUM (bitcast), halving
    PSUM-drain copy cost; output returned as bf16, host casts to f32.

Sharding: pure batch parallel, 2 batches per core on 8 cores.
"""

import os
import sys

import numpy as np

for _p in ("/opt/trn_rl_repo", "/root/.axon_site/_ro/trn_rl_repo"):
    if os.path.isdir(_p) and _p not in sys.path:
        sys.path.append(_p)

import concourse.bass as bass
import concourse.tile as tile
from concourse import bacc, masks, mybir
from concourse.bass_utils import run_bass_kernel_spmd

N, C, H, W = 16, 128, 48, 48
HW = H * W
NCORES = 8
NB = N // NCORES
SCALE = float(1.0 / (np.sqrt(np.float32(C)) * np.sqrt(np.float32(HW))))

F32 = mybir.dt.float32
BF16 = mybir.dt.bfloat16

CHUNKS512 = [(c0, min(512, HW - c0)) for c0 in range(0, HW, 512)]
HALVES = [(0, HW // 2), (HW // 2, HW // 2)]  # 1152 = 24 rows = 9 blocks
MINOP = mybir.AluOpType.min
MAXOP = mybir.AluOpType.max
MULOP = mybir.AluOpType.mult

BF16_PSUM = False  # matmul outputs must be fp32 in PSUM (HW constraint)


def build_kernel(tc, out_d, x_d, p5_d, w6_d, w8_d, w10_d):
    nc = tc.nc

    with (
        tc.tile_pool(name="const", bufs=1) as cpool,
        tc.tile_pool(name="prep", bufs=1) as prep,
        tc.tile_pool(name="batch", bufs=1) as bpool,
        tc.tile_pool(name="ps_big", bufs=4, space="PSUM") as ps_big,
        tc.tile_pool(name="ps_small", bufs=2, space="PSUM") as ps_small,
        tc.tile_pool(name="ps_G", bufs=2, space="PSUM") as ps_g,
    ):
        # ---- G queue: identity, x half-loads (SWDGE f32->bf16), memsets --
        ident = cpool.tile([128, 128], BF16, tag="ident")
        masks.make_identity(nc, ident[:])

        x_bf = [bpool.tile([C, HW], BF16, tag=f"x{b}", name=f"x{b}") for b in range(NB)]
        for b in range(NB):
            xsrc = x_d.ap()[b].rearrange("c h w -> c (h w)")
            for h0, hn in HALVES:
                nc.gpsimd.dma_start(x_bf[b][:, h0 : h0 + hn], xsrc[:, h0 : h0 + hn])

        # replicated padded t1 windows: t1p3[k] = grid rows [2k, 2k+48) x 60
        t1p3_tiles = []
        for b in range(NB):
            t1p3 = cpool.tile([3, 48 * 60], BF16, tag=f"t1p3{b}", name=f"t1p3{b}")
            nc.gpsimd.memset(t1p3[:], 0.0)
            t1p3_tiles.append(t1p3)

        # ---- sync queue: weight/p5 loads then x XBAR transposes -----------
        w6_sb = prep.tile([C, C], F32, tag="w6sb")
        nc.sync.dma_start(w6_sb[:], w6_d.ap())
        w8_sb = prep.tile([C // 2, 5], F32, tag="w8sb")
        nc.sync.dma_start(w8_sb[:], w8_d.ap()[:, 0, 0, :])
        w10_sb = prep.tile([C, 3 * C // 2], F32, tag="w10sb")
        nc.sync.dma_start(w10_sb[:], w10_d.ap())
        p5_sb = prep.tile([C, HW], F32, tag="p5sb")
        nc.sync.dma_start(p5_sb[:], p5_d.ap()[0].rearrange("c h w -> c (h w)"))

        # block-transposed x via XBAR: xT[q, 128i+c] = x[c, 128i+q]
        xT = [bpool.tile([C, HW], BF16, tag=f"xT{b}", name=f"xT{b}") for b in range(NB)]
        for b in range(NB):
            xTv = xT[b][:].rearrange("q (i c) -> q i c", c=128)
            for hi, (h0, hn) in enumerate(HALVES):
                nc.sync.dma_start_transpose(
                    xTv[:, hi * 9 : hi * 9 + 9, :], x_bf[b][:, h0 : h0 + hn]
                )

        # ---- S: bf16 weight casts; PE: w6T + K_eff prep -------------------
        w6_bf = prep.tile([C, C], BF16, tag="w6bf")
        nc.scalar.copy(w6_bf[:], w6_sb[:])
        w10_bf = prep.tile([C, 3 * C // 2], BF16, tag="w10bf")
        nc.scalar.copy(w10_bf[:], w10_sb[:])
        w8_bf = prep.tile([C // 2, 5], BF16, tag="w8bf")
        nc.scalar.copy(w8_bf[:], w8_sb[:])

        # w6T[c1, c'] = w6[c', c1]  (stationary for B = w6 G)
        psw = ps_small.tile([C, C], F32, tag="small")
        pswv = psw[:].bitcast(BF16)
        nc.tensor.transpose(pswv[:, 0:C], w6_bf[:], ident)
        w6T = cpool.tile([C, C], BF16, tag="w6T")
        nc.scalar.copy(w6T[:], pswv[:, 0:C])

        w10v = w10_bf[:].rearrange("c (m k) -> c k m", k=3)
        pk = ps_small.tile([5, 3 * C], F32, tag="small")
        for k in range(3):
            psk = ps_small.tile([C // 2, C], F32, tag="small")
            pskv = psk[:].bitcast(BF16)
            nc.tensor.transpose(pskv[:, 0:C], w10v[:, k, :], ident)
            w10Tk = prep.tile([C // 2, C], BF16, tag=f"w10T{k}")
            nc.scalar.copy(w10Tk[:, 1:C], pskv[:, 0 : C - 1])
            nc.scalar.copy(w10Tk[:, 0:1], pskv[:, C - 1 : C])
            nc.tensor.matmul(
                pk[:, k * C : (k + 1) * C], w8_bf[:], w10Tk[:], start=True, stop=True
            )
        keff_tmp = prep.tile([5, 3 * C], BF16, tag="kefftmp")
        nc.scalar.copy(keff_tmp[:], pk[:])
        K_effT = cpool.tile([15, C], BF16, tag="KeffT")
        nc.scalar.dma_start(K_effT[:], keff_tmp[:].rearrange("j (k o) -> j k o", k=3))

        # ---- per-batch tiles ----------------------------------------------
        def bt(shape, dt, nm):
            return [
                bpool.tile(shape, dt, tag=f"{nm}{b}", name=f"{nm}{b}")
                for b in range(NB)
            ]

        tmin_l = bt([C, HW], BF16, "tmin")
        t5_l = bt([C, HW], BF16, "t5")
        t5T_l = bt([C, HW], BF16, "t5T")
        t12T_l = bt([C, HW], BF16, "t12T")
        t1pk_l = bt([C, 18], BF16, "t1pk")
        t1row_l = bt([18, C], BF16, "t1row")
        sh_l = bt([15, HW], BF16, "sh")
        osb_l = bt([C, HW], BF16, "osb")

        def tmin_half(eng, b, hi):
            x3 = x_bf[b][:].rearrange("c (h w) -> c h w", w=W)
            tm3 = tmin_l[b][:].rearrange("c (h w) -> c h w", w=W)
            r0, r1 = (0, 24) if hi == 0 else (24, 48)
            eng.tensor_tensor(
                tm3[:, r0:r1, 1:W], x3[:, r0:r1, 1:W], x3[:, r0:r1, 0 : W - 1], MINOP
            )
            eng.tensor_tensor(
                tm3[:, r0:r1, 0:1], x3[:, r0:r1, 0:1], x3[:, r0:r1, W - 1 : W], MINOP
            )

        def t5_half(eng, b, hi):
            h0, hn = HALVES[hi]
            eng.scalar_tensor_tensor(
                t5_l[b][:, h0 : h0 + hn],
                tmin_l[b][:, h0 : h0 + hn],
                -1.0,
                p5_sb[:, h0 : h0 + hn],
                MULOP,
                MULOP,
            )

        def reduce_half(b, hi):
            xTv = xT[b][:].rearrange("q (i c) -> q i c", c=128)
            nc.vector.reduce_max(
                t1pk_l[b][:, hi * 9 : hi * 9 + 9],
                xTv[:, hi * 9 : hi * 9 + 9, :],
                axis=mybir.AxisListType.X,
            )

        def pst_transpose(b):
            pst = ps_small.tile([18, C], F32, tag="small")
            pstv = pst[:].bitcast(BF16)
            nc.tensor.transpose(pstv[:, 0:C], t1pk_l[b][:], ident)
            return pstv

        def chain_dmas(q, b):
            """t1row -> t1flat -> 3 window interiors -> 5 shifts -> sh."""
            v = t1p3_tiles[b][:].rearrange("p (r c) -> p r c", c=60)
            t1flat = bpool.tile([1, HW], BF16, tag=f"t1flat{b}", name=f"t1flat{b}")
            q.dma_start(t1flat[:], t1row_l[b][:])
            # window k local row l holds t1 row l + 2k - 2 (zero-padded)
            q.dma_start(v[0:1, 2:48, 6:54], t1flat[:, 0 : 46 * 48])
            q.dma_start(v[1:2, 0:48, 6:54], t1flat[:])
            q.dma_start(v[2:3, 0:46, 6:54], t1flat[:, 2 * 48 : HW])
            for j in range(5):
                q.dma_start(
                    sh_l[b][3 * j : 3 * j + 3, :], v[:, 0:48, 3 * j : 3 * j + 48]
                )

        def shot_ps():
            """PSUM tile for a single-shot matmul, bf16 view if enabled."""
            ps = ps_big.tile([C, 512], F32, tag="ps")
            return ps[:].bitcast(BF16) if BF16_PSUM else ps[:]

        def t11T_mm(b, chunks):
            for ci in chunks:
                c0, cn = CHUNKS512[ci]
                ps11 = shot_ps()
                for j in range(cn // 128):
                    col = c0 + j * 128
                    nc.tensor.matmul(
                        ps11[:, j * 128 : (j + 1) * 128],
                        sh_l[b][:, col : col + 128],
                        K_effT[:],
                        start=True,
                        stop=True,
                    )
                yield ci, ps11

        # ================= emission schedule ==============================
        # V (bottleneck ~15us): tmin/t5/reduces both batches, maxes both
        # S: casts, K_eff prep copies, t1rows, chain-b0, G/B muls, out-copies
        # sync: loads, xT XBARs, t5T-b0 XBARs, chain-b1
        # G (pool): ident, x loads, memsets, t5T-b1 XBAR?no(SWDGE) -> out DMAs
        # PE: prep, pst-b0, pst-b1, t11T-b0, G-b0, t11T-b1, B0/out-b0,
        #     G-b1, B1/out-b1

        # V: b0 tmin/reduce/t5, then b1 (in x-landing order)
        tmin_half(nc.vector, 0, 0)
        reduce_half(0, 0)
        tmin_half(nc.vector, 0, 1)
        reduce_half(0, 1)
        t5_half(nc.vector, 0, 0)
        t5_half(nc.vector, 0, 1)
        tmin_half(nc.vector, 1, 0)
        reduce_half(1, 0)
        tmin_half(nc.vector, 1, 1)
        reduce_half(1, 1)
        t5_half(nc.vector, 1, 0)
        t5_half(nc.vector, 1, 1)

        # PE: pst-b0; S: t1row-b0 + chain-b0 (t1flat + interiors + shifts)
        pstv0 = pst_transpose(0)
        nc.scalar.copy(t1row_l[0][:], pstv0[:, 0:C])
        chain_dmas(nc.scalar, 0)

        # sync: t5T-b0 XBAR halves (after t5-b0)
        t5Tv0 = t5T_l[0][:].rearrange("q (i c) -> q i c", c=128)
        for hi, (h0, hn) in enumerate(HALVES):
            nc.sync.dma_start_transpose(
                t5Tv0[:, hi * 9 : hi * 9 + 9, :], t5_l[0][:, h0 : h0 + hn]
            )

        # PE: pst-b1; S: t1row-b1; sync: chain-b1 then t5T-b1 XBAR
        pstv1 = pst_transpose(1)
        nc.scalar.copy(t1row_l[1][:], pstv1[:, 0:C])
        chain_dmas(nc.sync, 1)
        t5Tv1 = t5T_l[1][:].rearrange("q (i c) -> q i c", c=128)
        for hi, (h0, hn) in enumerate(HALVES):
            nc.sync.dma_start_transpose(
                t5Tv1[:, hi * 9 : hi * 9 + 9, :], t5_l[1][:, h0 : h0 + hn]
            )

        # PE: t11T-b0; V: t12T maxes b0
        for ci, ps11 in t11T_mm(0, range(5)):
            c0, cn = CHUNKS512[ci]
            nc.vector.tensor_tensor(
                t12T_l[0][:, c0 : c0 + cn],
                xT[0][:, c0 : c0 + cn],
                ps11[:, :cn],
                MAXOP,
            )

        # PE: G-b0 accumulation; S: Gs-b0 (plain bf16 copy)
        psG0 = ps_g.tile([C, C], F32, tag="G")
        for i in range(18):
            nc.tensor.matmul(
                psG0[:],
                t5T_l[0][:, i * 128 : (i + 1) * 128],
                t12T_l[0][:, i * 128 : (i + 1) * 128],
                start=(i == 0),
                stop=(i == 17),
            )
        Gs0 = bpool.tile([C, C], BF16, tag="Gs0")
        nc.scalar.copy(Gs0[:], psG0[:])

        # PE: t11T-b1; V: t12T maxes b1
        for ci, ps11 in t11T_mm(1, range(5)):
            c0, cn = CHUNKS512[ci]
            nc.vector.tensor_tensor(
                t12T_l[1][:, c0 : c0 + cn],
                xT[1][:, c0 : c0 + cn],
                ps11[:, :cn],
                MAXOP,
            )

        # PE: B0 = w6 G0 (scaled on drain); out-b0 = Bs0^T x0
        psB0 = ps_small.tile([C, C], F32, tag="small")
        nc.tensor.matmul(psB0[:], w6T[:], Gs0[:], start=True, stop=True)
        Bs0 = bpool.tile([C, C], BF16, tag="Bs0")
        nc.scalar.mul(Bs0[:], psB0[:], SCALE)
        for c0, cn in CHUNKS512:
            pso = shot_ps()
            nc.tensor.matmul(pso[:, :cn], Bs0[:], x_bf[0][:, c0 : c0 + cn],
                             start=True, stop=True)
            nc.vector.tensor_copy(osb_l[0][:, c0 : c0 + cn], pso[:, :cn])
        out0 = out_d.ap()[0].rearrange("c h w -> c (h w)")
        for h0, hn in HALVES:
            nc.gpsimd.dma_start(out0[:, h0 : h0 + hn], osb_l[0][:, h0 : h0 + hn])

        # PE: G-b1; S: Gs-b1; B1; out-b1
        psG1 = ps_g.tile([C, C], F32, tag="G")
        for i in range(18):
            nc.tensor.matmul(
                psG1[:],
                t5T_l[1][:, i * 128 : (i + 1) * 128],
                t12T_l[1][:, i * 128 : (i + 1) * 128],
                start=(i == 0),
                stop=(i == 17),
            )
        Gs1 = bpool.tile([C, C], BF16, tag="Gs1")
        nc.scalar.copy(Gs1[:], psG1[:])
        psB1 = ps_small.tile([C, C], F32, tag="small")
        nc.tensor.matmul(psB1[:], w6T[:], Gs1[:], start=True, stop=True)
        Bs1 = bpool.tile([C, C], BF16, tag="Bs1")
        nc.scalar.mul(Bs1[:], psB1[:], SCALE)
        for c0, cn in CHUNKS512:
            pso = shot_ps()
            nc.tensor.matmul(pso[:, :cn], Bs1[:], x_bf[1][:, c0 : c0 + cn],
                             start=True, stop=True)
            nc.scalar.copy(osb_l[1][:, c0 : c0 + cn], pso[:, :cn])
        out1 = out_d.ap()[1].rearrange("c h w -> c (h w)")
        for h0, hn in HALVES:
            nc.gpsimd.dma_start(out1[:, h0 : h0 + hn], osb_l[1][:, h0 : h0 + hn])


def build_bass():
    nc = bacc.Bacc("TRN2", target_bir_lowering=False, debug=False, num_devices=NCORES)
    x_d = nc.dram_tensor("x", [NB, C, H, W], F32, kind="ExternalInput")
    p5_d = nc.dram_tensor("p5_w", [1, C, H, W], F32, kind="ExternalInput")
    w6_d = nc.dram_tensor("w6", [C, C], F32, kind="ExternalInput")
    w8_d = nc.dram_tensor("w8", [C // 2, 1, 1, 5], F32, kind="ExternalInput")
    w10_d = nc.dram_tensor("w10", [C, 3 * C // 2], F32, kind="ExternalInput")
    out_d = nc.dram_tensor("out", [NB, C, H, W], BF16, kind="ExternalOutput")
    with tile.TileContext(nc) as tc:
        build_kernel(tc, out_d, x_d, p5_d, w6_d, w8_d, w10_d)
    nc.compile()
    return nc


_NC_CACHE = {}


def _get_nc():
    if "nc" not in _NC_CACHE:
        _NC_CACHE["nc"] = build_bass()
    return _NC_CACHE["nc"]


def kernel(x, p5_w, w6, w8, w10, trace=False, trace_kwargs=None):
    x = np.ascontiguousarray(x, dtype=np.float32)
    nc = _get_nc()
    in_maps = []
    for core in range(NCORES):
        in_maps.append(
            {
                "x": x[core * NB : (core + 1) * NB],
                "p5_w": np.asarray(p5_w, dtype=np.float32),
                "w6": np.asarray(w6, dtype=np.float32),
                "w8": np.asarray(w8, dtype=np.float32),
                "w10": np.asarray(w10, dtype=np.float32),
            }
        )
    res = run_bass_kernel_spmd(
        nc,
        in_maps,
        list(range(NCORES)),
        trace=trace,
        **(trace_kwargs or {}),
    )
    out = np.concatenate(
        [np.asarray(res.results[i]["out"], dtype=np.float32) for i in range(NCORES)],
        axis=0,
    )
    if trace:
        return out, res
    return out


# revision 16
# speedup vs baseline: 1.3879x; 1.3879x over previous
"""Trainium2 Bass kernel for nn_Kernel_6199962355332830965 (sparse_attention).

Reference computation (per batch n, with C=128, H=W=48, HW=2304):
    t1  = max_c x ;  t4 = max(-x, roll(-x,1,w)) = -min(x, roll_w(x))
    t5  = p5_w * t4 ;  t6 = w6 @ t5
    t7  = t6^T x / sqrt(C)
    t8/t9/t10/t11 = dilated conv + unfold + 1x1 conv + channel roll of t1
    t12 = max(x, t11)
    out[n,q,c] = sum_p t7[n,p,q] t12[n,c,p] / sqrt(HW)

Algebraic restructuring (v4):
    out = s * B^T x  with  B[c',c] = sum_p t6[c',p] t12[c,p]
  - t6T produced position-major directly by fused-transpose matmuls
    (A = t5 chunk stationary, moving = w6T), xT by the DMA XBAR.
  - t12T = max(xT, t11T); t11T from matmuls of 15 shifted copies of
    zero-padded t1 against K_effT[(3j+k),o] = sum_m w8[m,j]
    w10[(o-1)%C, 3m+k] (conv+unfold+1x1+roll in one 15-tap kernel).
  - p5 pre-negated to bf16 on Scalar so t5 = tmin * p5n is a plain
    bf16 tensor_tensor; output returned bf16, host casts to f32.
  - chain DMAs and XBARs split across the SP/Act queues by need-time;
    V carries all binary elementwise (Pool cannot run tensor_tensor).

Sharding: pure batch parallel, 2 batches per core on 8 cores.
"""

import os
import sys

import numpy as np

for _p in ("/opt/trn_rl_repo", "/root/.axon_site/_ro/trn_rl_repo"):
    if os.path.isdir(_p) and _p not in sys.path:
        sys.path.append(_p)

import concourse.bass as bass
import concourse.tile as tile
from concourse import bacc, masks, mybir
from concourse.bass_utils import run_bass_kernel_spmd

N, C, H, W = 16, 128, 48, 48
HW = H * W
NCORES = 8
NB = N // NCORES
SCALE = float(1.0 / (np.sqrt(np.float32(C)) * np.sqrt(np.float32(HW))))

F32 = mybir.dt.float32
BF16 = mybir.dt.bfloat16

CHUNKS512 = [(c0, min(512, HW - c0)) for c0 in range(0, HW, 512)]
HALVES = [(0, HW // 2), (HW // 2, HW // 2)]  # 1152 = 24 rows = 9 blocks
MINOP = mybir.AluOpType.min
MAXOP = mybir.AluOpType.max
MULOP = mybir.AluOpType.mult

BF16_PSUM = False  # matmul outputs must be fp32 in PSUM (HW constraint)


def build_kernel(tc, out_d, x_d, p5_d, w6_d, w8_d, w10_d):
    nc = tc.nc

    with (
        tc.tile_pool(name="const", bufs=1) as cpool,
        tc.tile_pool(name="prep", bufs=1) as prep,
        tc.tile_pool(name="batch", bufs=1) as bpool,
        tc.tile_pool(name="ps_big", bufs=4, space="PSUM") as ps_big,
        tc.tile_pool(name="ps_small", bufs=2, space="PSUM") as ps_small,
        tc.tile_pool(name="ps_G", bufs=2, space="PSUM") as ps_g,
    ):
        # ---- G queue: identity, x half-loads (SWDGE f32->bf16), memsets --
        ident = cpool.tile([128, 128], BF16, tag="ident")
        masks.make_identity(nc, ident[:])

        x_bf = [bpool.tile([C, HW], BF16, tag=f"x{b}", name=f"x{b}") for b in range(NB)]
        for b in range(NB):
            xsrc = x_d.ap()[b].rearrange("c h w -> c (h w)")
            for h0, hn in HALVES:
                nc.gpsimd.dma_start(x_bf[b][:, h0 : h0 + hn], xsrc[:, h0 : h0 + hn])

        # replicated padded t1 windows: t1p3[k] = grid rows [2k, 2k+48) x 60
        t1p3_tiles = []
        for b in range(NB):
            t1p3 = cpool.tile([3, 48 * 60], BF16, tag=f"t1p3{b}", name=f"t1p3{b}")
            nc.gpsimd.memset(t1p3[:], 0.0)
            t1p3_tiles.append(t1p3)

        # ---- sync queue: weight/p5 loads then x XBAR transposes -----------
        w6_sb = prep.tile([C, C], F32, tag="w6sb")
        nc.sync.dma_start(w6_sb[:], w6_d.ap())
        w8_sb = prep.tile([C // 2, 5], F32, tag="w8sb")
        nc.sync.dma_start(w8_sb[:], w8_d.ap()[:, 0, 0, :])
        w10_sb = prep.tile([C, 3 * C // 2], F32, tag="w10sb")
        nc.sync.dma_start(w10_sb[:], w10_d.ap())
        p5_sb = prep.tile([C, HW], F32, tag="p5sb")
        nc.sync.dma_start(p5_sb[:], p5_d.ap()[0].rearrange("c h w -> c (h w)"))

        # block-transposed x via XBAR: xT[q, 128i+c] = x[c, 128i+q]
        xT = [bpool.tile([C, HW], BF16, tag=f"xT{b}", name=f"xT{b}") for b in range(NB)]
        for b in range(NB):
            xTv = xT[b][:].rearrange("q (i c) -> q i c", c=128)
            for hi, (h0, hn) in enumerate(HALVES):
                nc.sync.dma_start_transpose(
                    xTv[:, hi * 9 : hi * 9 + 9, :], x_bf[b][:, h0 : h0 + hn]
                )

        # ---- S: bf16 weight casts; PE: w6T + K_eff prep -------------------
        w6_bf = prep.tile([C, C], BF16, tag="w6bf")
        nc.scalar.copy(w6_bf[:], w6_sb[:])
        w10_bf = prep.tile([C, 3 * C // 2], BF16, tag="w10bf")
        nc.scalar.copy(w10_bf[:], w10_sb[:])
        w8_bf = prep.tile([C // 2, 5], BF16, tag="w8bf")
        nc.scalar.copy(w8_bf[:], w8_sb[:])

        # w6T[c1, c'] = w6[c', c1]  (stationary for B = w6 G)
        psw = ps_small.tile([C, C], F32, tag="small")
        pswv = psw[:].bitcast(BF16)
        nc.tensor.transpose(pswv[:, 0:C], w6_bf[:], ident)
        w6T = cpool.tile([C, C], BF16, tag="w6T")
        nc.scalar.copy(w6T[:], pswv[:, 0:C])

        # p5n = -p5 in bf16 (Scalar) so t5 is a cheap bf16 tensor_tensor
        p5n = prep.tile([C, HW], BF16, tag="p5n")
        nc.scalar.mul(p5n[:], p5_sb[:], -1.0)

        w10v = w10_bf[:].rearrange("c (m k) -> c k m", k=3)
        pk = ps_small.tile([5, 3 * C], F32, tag="small")
        for k in range(3):
            psk = ps_small.tile([C // 2, C], F32, tag="small")
            pskv = psk[:].bitcast(BF16)
            nc.tensor.transpose(pskv[:, 0:C], w10v[:, k, :], ident)
            w10Tk = prep.tile([C // 2, C], BF16, tag=f"w10T{k}")
            nc.scalar.copy(w10Tk[:, 1:C], pskv[:, 0 : C - 1])
            nc.scalar.copy(w10Tk[:, 0:1], pskv[:, C - 1 : C])
            nc.tensor.matmul(
                pk[:, k * C : (k + 1) * C], w8_bf[:], w10Tk[:], start=True, stop=True
            )
        keff_tmp = prep.tile([5, 3 * C], BF16, tag="kefftmp")
        nc.scalar.copy(keff_tmp[:], pk[:])
        K_effT = cpool.tile([15, C], BF16, tag="KeffT")
        nc.scalar.dma_start(K_effT[:], keff_tmp[:].rearrange("j (k o) -> j k o", k=3))

        # ---- per-batch tiles ----------------------------------------------
        def bt(shape, dt, nm):
            return [
                bpool.tile(shape, dt, tag=f"{nm}{b}", name=f"{nm}{b}")
                for b in range(NB)
            ]

        tmin_l = bt([C, HW], BF16, "tmin")
        t5_l = bt([C, HW], BF16, "t5")
        t6T_l = bt([C, HW], BF16, "t6T")
        t12T_l = bt([C, HW], BF16, "t12T")
        t1pk_l = bt([C, 18], BF16, "t1pk")
        t1row_l = bt([18, C], BF16, "t1row")
        sh_l = bt([15, HW], BF16, "sh")
        osb_l = bt([C, HW], BF16, "osb")

        def tmin_half(eng, b, hi):
            x3 = x_bf[b][:].rearrange("c (h w) -> c h w", w=W)
            tm3 = tmin_l[b][:].rearrange("c (h w) -> c h w", w=W)
            r0, r1 = (0, 24) if hi == 0 else (24, 48)
            eng.tensor_tensor(
                tm3[:, r0:r1, 1:W], x3[:, r0:r1, 1:W], x3[:, r0:r1, 0 : W - 1], MINOP
            )
            eng.tensor_tensor(
                tm3[:, r0:r1, 0:1], x3[:, r0:r1, 0:1], x3[:, r0:r1, W - 1 : W], MINOP
            )

        def t5_half(eng, b, hi):
            h0, hn = HALVES[hi]
            eng.tensor_tensor(
                t5_l[b][:, h0 : h0 + hn],
                tmin_l[b][:, h0 : h0 + hn],
                p5n[:, h0 : h0 + hn],
                MULOP,
            )

        def reduce_half(b, hi):
            xTv = xT[b][:].rearrange("q (i c) -> q i c", c=128)
            nc.vector.reduce_max(
                t1pk_l[b][:, hi * 9 : hi * 9 + 9],
                xTv[:, hi * 9 : hi * 9 + 9, :],
                axis=mybir.AxisListType.X,
            )

        def pst_transpose(b):
            pst = ps_small.tile([18, C], F32, tag="small")
            pstv = pst[:].bitcast(BF16)
            nc.tensor.transpose(pstv[:, 0:C], t1pk_l[b][:], ident)
            return pstv

        def chain_head(q, b):
            """t1row -> t1flat -> 3 window interiors (on queue q)."""
            v = t1p3_tiles[b][:].rearrange("p (r c) -> p r c", c=60)
            t1flat = bpool.tile([1, HW], BF16, tag=f"t1flat{b}", name=f"t1flat{b}")
            q.dma_start(t1flat[:], t1row_l[b][:])
            # window k local row l holds t1 row l + 2k - 2 (zero-padded)
            q.dma_start(v[0:1, 2:48, 6:54], t1flat[:, 0 : 46 * 48])
            q.dma_start(v[1:2, 0:48, 6:54], t1flat[:])
            q.dma_start(v[2:3, 0:46, 6:54], t1flat[:, 2 * 48 : HW])

        def chain_shifts(q, b, js):
            v = t1p3_tiles[b][:].rearrange("p (r c) -> p r c", c=60)
            for j in js:
                q.dma_start(
                    sh_l[b][3 * j : 3 * j + 3, :], v[:, 0:48, 3 * j : 3 * j + 48]
                )

        def shot_ps():
            """PSUM tile for a single-shot matmul, bf16 view if enabled."""
            ps = ps_big.tile([C, 512], F32, tag="ps")
            return ps[:].bitcast(BF16) if BF16_PSUM else ps[:]

        def t6T_mm(b, chunks):
            """t6T[q, 128i+o] = sum_c t5[c, 128i+q] w6T[c, o] (fused T)."""
            for ci in chunks:
                c0, cn = CHUNKS512[ci]
                ps6 = shot_ps()
                for j in range(cn // 128):
                    col = c0 + j * 128
                    nc.tensor.matmul(
                        ps6[:, j * 128 : (j + 1) * 128],
                        t5_l[b][:, col : col + 128],
                        w6T[:],
                        start=True,
                        stop=True,
                    )
                yield ci, ps6

        def t11T_mm(b, chunks):
            for ci in chunks:
                c0, cn = CHUNKS512[ci]
                ps11 = shot_ps()
                for j in range(cn // 128):
                    col = c0 + j * 128
                    nc.tensor.matmul(
                        ps11[:, j * 128 : (j + 1) * 128],
                        sh_l[b][:, col : col + 128],
                        K_effT[:],
                        start=True,
                        stop=True,
                    )
                yield ci, ps11

        # ================= emission schedule ==============================
        # V: tmin/reduce/t5 per batch in x-landing order, maxes, osb-b0
        # S: ACT prep, casts, w6T, Kprep, p5n, t1rows, chain-b0 head,
        #    shifts-b0 (2), t6T copies, shifts-b1 (3), Bs, osb-b1
        # sync: loads, xT XBARs, shifts-b0 (3), chain-b1 head, shifts-b1 (2)
        # G: ident, x loads, t1p3 memsets, out DMAs
        # PE: prep, pst-b0, t6T-b0, t11T-b0, pst-b1, B-b0, t6T-b1,
        #     t11T-b1, out-b0, B-b1, out-b1

        # V: b0 tmin/reduce/t5, then b1
        tmin_half(nc.vector, 0, 0)
        tmin_half(nc.vector, 0, 1)
        reduce_half(0, 0)
        reduce_half(0, 1)
        t5_half(nc.vector, 0, 0)
        t5_half(nc.vector, 0, 1)

        # PE: pst-b0; S: t1row-b0, chain-b0 head; shifts split S/sync
        pstv0 = pst_transpose(0)
        nc.scalar.copy(t1row_l[0][:], pstv0[:, 0:C])
        chain_head(nc.scalar, 0)
        chain_shifts(nc.sync, 0, [0, 1, 2])
        chain_shifts(nc.scalar, 0, [3, 4])

        # V: b1 tmin/reduce/t5
        tmin_half(nc.vector, 1, 0)
        tmin_half(nc.vector, 1, 1)
        reduce_half(1, 0)
        reduce_half(1, 1)
        t5_half(nc.vector, 1, 0)
        t5_half(nc.vector, 1, 1)

        # PE: t6T-b0; S copies
        t6ps0 = list(t6T_mm(0, range(5)))

        # PE: t11T-b0; V maxes b0
        ps11_0 = list(t11T_mm(0, range(5)))

        # PE: pst-b1; S: t1row-b1; sync: chain-b1 head; shifts split
        pstv1 = pst_transpose(1)
        nc.scalar.copy(t1row_l[1][:], pstv1[:, 0:C])
        chain_head(nc.sync, 1)
        chain_shifts(nc.scalar, 1, [0, 1, 2])
        chain_shifts(nc.sync, 1, [3, 4])

        # S: t6T-b0 copies (gate B-b0)
        for ci, ps6 in t6ps0:
            c0, cn = CHUNKS512[ci]
            nc.scalar.copy(t6T_l[0][:, c0 : c0 + cn], ps6[:, :cn])

        # V: maxes b0
        for ci, ps11 in ps11_0:
            c0, cn = CHUNKS512[ci]
            nc.vector.tensor_tensor(
                t12T_l[0][:, c0 : c0 + cn],
                xT[0][:, c0 : c0 + cn],
                ps11[:, :cn],
                MAXOP,
            )

        # PE: B-b0
        psB0 = ps_g.tile([C, C], F32, tag="B")
        for i in range(18):
            nc.tensor.matmul(
                psB0[:],
                t6T_l[0][:, i * 128 : (i + 1) * 128],
                t12T_l[0][:, i * 128 : (i + 1) * 128],
                start=(i == 0),
                stop=(i == 17),
            )
        Bs0 = bpool.tile([C, C], BF16, tag="Bs0")
        nc.scalar.mul(Bs0[:], psB0[:], SCALE)

        # PE: t6T-b1; S copies (gate B-b1)
        for ci, ps6 in t6T_mm(1, range(5)):
            c0, cn = CHUNKS512[ci]
            nc.scalar.copy(t6T_l[1][:, c0 : c0 + cn], ps6[:, :cn])

        # PE: t11T-b1; V maxes b1
        for ci, ps11 in t11T_mm(1, range(5)):
            c0, cn = CHUNKS512[ci]
            nc.vector.tensor_tensor(
                t12T_l[1][:, c0 : c0 + cn],
                xT[1][:, c0 : c0 + cn],
                ps11[:, :cn],
                MAXOP,
            )

        # PE: out-b0; V copies osb-b0; G DMAs out
        for c0, cn in CHUNKS512:
            pso = shot_ps()
            nc.tensor.matmul(pso[:, :cn], Bs0[:], x_bf[0][:, c0 : c0 + cn],
                             start=True, stop=True)
            nc.vector.tensor_copy(osb_l[0][:, c0 : c0 + cn], pso[:, :cn])
        out0 = out_d.ap()[0].rearrange("c h w -> c (h w)")
        for h0, hn in HALVES:
            nc.gpsimd.dma_start(out0[:, h0 : h0 + hn], osb_l[0][:, h0 : h0 + hn])

        # PE: B-b1; S: Bs-b1; out-b1; S copies osb-b1; G DMAs out
        psB1 = ps_g.tile([C, C], F32, tag="B")
        for i in range(18):
            nc.tensor.matmul(
                psB1[:],
                t6T_l[1][:, i * 128 : (i + 1) * 128],
                t12T_l[1][:, i * 128 : (i + 1) * 128],
                start=(i == 0),
                stop=(i == 17),
            )
        Bs1 = bpool.tile([C, C], BF16, tag="Bs1")
        nc.scalar.mul(Bs1[:], psB1[:], SCALE)
        for c0, cn in CHUNKS512:
            pso = shot_ps()
            nc.tensor.matmul(pso[:, :cn], Bs1[:], x_bf[1][:, c0 : c0 + cn],
                             start=True, stop=True)
            nc.scalar.copy(osb_l[1][:, c0 : c0 + cn], pso[:, :cn])
        out1 = out_d.ap()[1].rearrange("c h w -> c (h w)")
        for h0, hn in HALVES:
            nc.gpsimd.dma_start(out1[:, h0 : h0 + hn], osb_l[1][:, h0 : h0 + hn])


def build_bass():
    nc = bacc.Bacc("TRN2", target_bir_lowering=False, debug=False, num_devices=NCORES)
    x_d = nc.dram_tensor("x", [NB, C, H, W], F32, kind="ExternalInput")
    p5_d = nc.dram_tensor("p5_w", [1, C, H, W], F32, kind="ExternalInput")
    w6_d = nc.dram_tensor("w6", [C, C], F32, kind="ExternalInput")
    w8_d = nc.dram_tensor("w8", [C // 2, 1, 1, 5], F32, kind="ExternalInput")
    w10_d = nc.dram_tensor("w10", [C, 3 * C // 2], F32, kind="ExternalInput")
    out_d = nc.dram_tensor("out", [NB, C, H, W], BF16, kind="ExternalOutput")
    with tile.TileContext(nc) as tc:
        build_kernel(tc, out_d, x_d, p5_d, w6_d, w8_d, w10_d)
    nc.compile()
    return nc


_NC_CACHE = {}


def _get_nc():
    if "nc" not in _NC_CACHE:
        _NC_CACHE["nc"] = build_bass()
    return _NC_CACHE["nc"]


def kernel(x, p5_w, w6, w8, w10, trace=False, trace_kwargs=None):
    x = np.ascontiguousarray(x, dtype=np.float32)
    nc = _get_nc()
    in_maps = []
    for core in range(NCORES):
        in_maps.append(
            {
                "x": x[core * NB : (core + 1) * NB],
                "p5_w": np.asarray(p5_w, dtype=np.float32),
                "w6": np.asarray(w6, dtype=np.float32),
                "w8": np.asarray(w8, dtype=np.float32),
                "w10": np.asarray(w10, dtype=np.float32),
            }
        )
    res = run_bass_kernel_spmd(
        nc,
        in_maps,
        list(range(NCORES)),
        trace=trace,
        **(trace_kwargs or {}),
    )
    out = np.concatenate(
        [np.asarray(res.results[i]["out"], dtype=np.float32) for i in range(NCORES)],
        axis=0,
    )
    if trace:
        return out, res
    return out
